# revision 14
# baseline (speedup 1.0000x reference)
"""CQAttention Trainium2 kernel.

Math (per batch b, H=256, q=2048, d=8192):
  Qp   = gelu(Q @ W.T + b)                       [q, H]
  S    = C @ Qp.T                                [d, q]
  P    = softmax(S, axis=q)
  out  = P @ Qp + C                              [d, H]

Sharding: data-parallel over batch, one batch per NeuronCore (8 cores).

Per-core pipeline (all matmuls contract over the feature dim or q):
  - Q^T, W^T via PE transposes; QpT = gelu(W Q^T + b) with per-partition bias
    on the ACT engine; Qp (natural, bf16) by transposing QpT back, augmented
    with a ones column so the softmax denominator falls out of the second
    matmul's PSUM accumulation.
  - Per 512-row chunk of C: transpose C tiles to put the feature dim on
    partitions; logits^T tiles [q=128, d=512] in fp32r (full PE rate for
    N>=256, full fp32 operand precision); exp on ACT straight from PSUM to
    bf16 (softmax without max-subtraction: |logits| < ~70 so fp32 exp is
    safe); attended accumulated over the 16 q-tiles into PSUM [d=128, 257]
    where column 256 is the row-sum; fused epilogue
    out = (attended * 1/rowsum) + C in one DVE op per tile.
"""

from contextlib import ExitStack

import numpy as np

import concourse.bass as bass
import concourse.mybir as mybir
import concourse.tile as tile
from concourse import bacc
from concourse.bass_utils import run_bass_kernel_spmd
from concourse.masks import make_identity

B, QL, D, H = 8, 2048, 8192, 256
N_CORES = 8
F32 = mybir.dt.float32
F32R = mybir.dt.float32r
BF16 = mybir.dt.bfloat16

HC = H // 128      # feature chunks (2)
NQT = QL // 128    # q tiles (16)
DC = 512           # d-chunk size
NDC = D // DC      # d chunks (16)
NDM = DC // 128    # d tiles per chunk (4)

# "f32r": store logits operands in f32, run matmul as float32r (full rate).
# "bf16": store logits operands as bf16.
LOGITS_DT = "f32r"

AF = mybir.ActivationFunctionType
ALU = mybir.AluOpType


LS = F32R if LOGITS_DT == "f32r" else BF16


def build_body(ctx: ExitStack, tc: tile.TileContext, nc, Qd, Cd, Wd, bd, Od):
    singles = ctx.enter_context(tc.tile_pool(name="singles", bufs=1))
    qstat = ctx.enter_context(tc.tile_pool(name="qstat", bufs=1))
    cpool = ctx.enter_context(tc.tile_pool(name="cpool", bufs=3))
    ctpool = ctx.enter_context(tc.tile_pool(name="ctp", bufs=3))
    exppool = ctx.enter_context(tc.tile_pool(name="expp", bufs=2))
    outpool = ctx.enter_context(tc.tile_pool(name="outp", bufs=3))
    small = ctx.enter_context(tc.tile_pool(name="small", bufs=4))
    psum_l = ctx.enter_context(tc.tile_pool(name="psl", bufs=2, space="PSUM"))
    psum_t = ctx.enter_context(tc.tile_pool(name="pst", bufs=2, space="PSUM"))
    psum_a = ctx.enter_context(tc.tile_pool(name="psa", bufs=1, space="PSUM"))

    ident = singles.tile([128, 128], F32)
    make_identity(nc, ident)

    # --- main loop over d chunks, software-pipelined C prep ---
    def c_load(dc):
        c_nat = cpool.tile([128, NDM, H], F32, tag="cnat", name=f"cnat{dc}")
        nc.sync.dma_start(
            out=c_nat[:],
            in_=Cd[dc * DC:(dc + 1) * DC, :].rearrange("(a p) h -> p a h", p=128))
        return c_nat

    def c_transpose(dc, c_nat, hc):
        pt = psum_t.tile([128, 512], F32, tag="pt", name=f"ptc{dc}_{hc}")
        for dm in range(NDM):
            nc.tensor.transpose(
                pt[:, dm * 128:(dm + 1) * 128],
                c_nat[:, dm, hc * 128:(hc + 1) * 128], ident[:])
        return pt

    c_nats = {0: c_load(0), 1: c_load(1)}
    cts = {}

    def c_prep(dc):
        cts[dc] = ctpool.tile([128, HC, DC], LS, tag="ct", name=f"ct{dc}")
        for hc in range(HC):
            pt = c_transpose(dc, c_nats[dc], hc)
            nc.vector.tensor_copy(cts[dc][:, hc, :], pt[:])

    # --- W^T [h, o] and bias ---
    w_nat = singles.tile([128, HC, H], F32)  # [o in-chunk, om, h]
    for om in range(HC):
        nc.sync.dma_start(out=w_nat[:, om, :], in_=Wd[om * 128:(om + 1) * 128, :])
    wt = qstat.tile([128, HC, H], LS)  # [h in-chunk, hc, o]
    for om in range(HC):
        for hc in range(HC):
            pt = psum_t.tile([128, 128], F32)
            nc.tensor.transpose(pt[:], w_nat[:, om, hc * 128:(hc + 1) * 128], ident[:])
            nc.vector.tensor_copy(wt[:, hc, om * 128:(om + 1) * 128], pt[:])
    bias = singles.tile([128, HC, 1], F32)
    nc.sync.dma_start(out=bias[:, :, 0], in_=bd.rearrange("(c p) -> p c", p=128))
    c_prep(0)

    # --- Q^T [h, q] ---
    qt = qstat.tile([128, HC, QL], LS)
    q_nat = cpool.tile([128, NQT, H], F32, tag="qnat")
    q_view = Qd.rearrange("(a p) h -> p a h", p=128)
    for qg in range(NQT // 4):
        nc.sync.dma_start(out=q_nat[:, qg * 4:(qg + 1) * 4, :],
                          in_=q_view[:, qg * 4:(qg + 1) * 4, :])
    for qg in range(NQT // 4):
        for hc in range(HC):
            pt = psum_t.tile([128, 512], F32)
            for k in range(4):
                qi = qg * 4 + k
                nc.tensor.transpose(
                    pt[:, k * 128:(k + 1) * 128],
                    q_nat[:, qi, hc * 128:(hc + 1) * 128], ident[:])
            nc.vector.tensor_copy(qt[:, hc, qg * 512:(qg + 1) * 512], pt[:])

    c_prep(1)

    # --- QpT = gelu(W Q^T + b) [o, q] ---
    qpt = qstat.tile([128, HC, QL], LS)
    for om in range(HC):
        for nn in range(QL // 512):
            pl = psum_l.tile([128, 512], F32)
            for hc in range(HC):
                nc.tensor.matmul(
                    pl[:],
                    wt[:, hc, om * 128:(om + 1) * 128],
                    qt[:, hc, nn * 512:(nn + 1) * 512],
                    start=(hc == 0),
                    stop=(hc == HC - 1),
                )
            nc.scalar.activation(
                qpt[:, om, nn * 512:(nn + 1) * 512], pl[:], AF.Gelu,
                bias=bias[:, om, :], scale=1.0,
            )

    # --- Qp natural bf16 [q, o] with ones column ---
    qp = qstat.tile([128, NQT, H + 1], BF16)
    for qg in range(NQT // 4):
        for om in range(HC):
            pt = psum_t.tile([128, 512], F32)
            for k in range(4):
                qi = qg * 4 + k
                src = qpt[:, om, qi * 128:(qi + 1) * 128]
                if LOGITS_DT == "f32r":
                    src = src.bitcast(F32)
                nc.tensor.transpose(pt[:, k * 128:(k + 1) * 128], src, ident[:])
            nc.vector.tensor_copy(
                qp[:, qg * 4:(qg + 1) * 4, om * 128:(om + 1) * 128],
                pt.rearrange("p (a b) -> p a b", a=4))
    nc.vector.memset(qp[:, :, H:H + 1], 1.0)

    # Lag the attended matmuls two q-tiles behind logits+exp so the PE
    # never waits on the ACT exp latency.
    LAG = 2
    for dc in range(NDC):
        c_nat, ct = c_nats[dc], cts[dc]
        expt = exppool.tile([128, NQT, DC], BF16)
        pa = [psum_a.tile([128, H + 1], F32, tag=f"a{dm}", name=f"pa{dm}")
              for dm in range(NDM)]
        nxt = dc + 2
        for step in range(NQT + LAG):
            if step == 4 and nxt < NDC:
                c_nats[nxt] = c_load(nxt)
                cts[nxt] = ctpool.tile([128, HC, DC], LS, tag="ct", name=f"ct{nxt}")
            if step in (8, 11) and nxt < NDC:
                hc = 0 if step == 8 else 1
                pt = c_transpose(nxt, c_nats[nxt], hc)
                nc.vector.tensor_copy(cts[nxt][:, hc, :], pt[:])
            if step < NQT:
                qi = step
                pl = psum_l.tile([128, DC], F32)
                for hc in range(HC):
                    nc.tensor.matmul(
                        pl[:],
                        qpt[:, hc, qi * 128:(qi + 1) * 128],
                        ct[:, hc, :],
                        start=(hc == 0),
                        stop=(hc == HC - 1),
                    )
                nc.scalar.activation(expt[:, qi, :], pl[:], AF.Exp)
            if step >= LAG:
                qj = step - LAG
                for dm in range(NDM):
                    nc.tensor.matmul(
                        pa[dm][:],
                        expt[:, qj, dm * 128:(dm + 1) * 128],
                        qp[:, qj, :],
                        start=(qj == 0),
                        stop=(qj == NQT - 1),
                    )

        o_sb = outpool.tile([128, NDM, H], F32)
        for dm in range(NDM):
            rec = small.tile([128, 1], F32)
            nc.vector.reciprocal(rec[:], pa[dm][:, H:H + 1])
            nc.vector.scalar_tensor_tensor(
                o_sb[:, dm, :], pa[dm][:, 0:H], rec[:], c_nat[:, dm, :],
                ALU.mult, ALU.add,
            )
        nc.sync.dma_start(
            out=Od[dc * DC:(dc + 1) * DC, :].rearrange("(a p) h -> p a h", p=128),
            in_=o_sb[:])
        del c_nats[dc], cts[dc]


def build_nc():
    nc = bacc.Bacc("TRN2", target_bir_lowering=False, debug=False,
                   num_devices=N_CORES)
    Qd = nc.dram_tensor("Q", [QL, H], F32, kind="ExternalInput")
    Cd = nc.dram_tensor("C", [D, H], F32, kind="ExternalInput")
    Wd = nc.dram_tensor("W", [H, H], F32, kind="ExternalInput")
    bd = nc.dram_tensor("b", [H], F32, kind="ExternalInput")
    Od = nc.dram_tensor("out", [D, H], F32, kind="ExternalOutput")
    with tile.TileContext(nc) as tc:
        with ExitStack() as ctx:
            build_body(ctx, tc, nc, Qd[:], Cd[:], Wd[:], bd[:], Od[:])
    nc.finalize()
    return nc


_NC = None


def get_nc():
    global _NC
    if _NC is None:
        _NC = build_nc()
    return _NC


def kernel(Q, C, W, b):
    assert Q.shape == (B, QL, H) and C.shape == (B, D, H)
    nc = get_nc()
    in_maps = [
        {
            "Q": np.ascontiguousarray(Q[i], dtype=np.float32),
            "C": np.ascontiguousarray(C[i], dtype=np.float32),
            "W": np.ascontiguousarray(W, dtype=np.float32),
            "b": np.ascontiguousarray(b, dtype=np.float32),
        }
        for i in range(N_CORES)
    ]
    res = run_bass_kernel_spmd(nc, in_maps, core_ids=list(range(N_CORES)))
    return np.stack([res.results[i]["out"] for i in range(N_CORES)], axis=0)


# revision 15
# speedup vs baseline: 1.0352x; 1.0352x over previous
"""CQAttention Trainium2 kernel.

Math (per batch b, H=256, q=2048, d=8192):
  Qp   = gelu(Q @ W.T + b)                       [q, H]
  S    = C @ Qp.T                                [d, q]
  P    = softmax(S, axis=q)
  out  = P @ Qp + C                              [d, H]

Sharding: data-parallel over batch, one batch per NeuronCore (8 cores).

Per-core pipeline (all matmuls contract over the feature dim or q):
  - Q^T, W^T via PE transposes; QpT = gelu(W Q^T + b) with per-partition bias
    on the ACT engine; Qp (natural, bf16) by transposing QpT back, augmented
    with a ones column so the softmax denominator falls out of the second
    matmul's PSUM accumulation.
  - Per 512-row chunk of C: transpose C tiles to put the feature dim on
    partitions; logits^T tiles [q=128, d=512] in fp32r (full PE rate for
    N>=256, full fp32 operand precision); exp on ACT straight from PSUM to
    bf16 (softmax without max-subtraction: |logits| < ~70 so fp32 exp is
    safe); attended accumulated over the 16 q-tiles into PSUM [d=128, 257]
    where column 256 is the row-sum; fused epilogue
    out = (attended * 1/rowsum) + C in one DVE op per tile.
"""

from contextlib import ExitStack

import numpy as np

import concourse.bass as bass
import concourse.mybir as mybir
import concourse.tile as tile
from concourse import bacc
from concourse.bass_utils import run_bass_kernel_spmd
from concourse.masks import make_identity

B, QL, D, H = 8, 2048, 8192, 256
N_CORES = 8
F32 = mybir.dt.float32
F32R = mybir.dt.float32r
BF16 = mybir.dt.bfloat16

HC = H // 128      # feature chunks (2)
NQT = QL // 128    # q tiles (16)
DC = 512           # d-chunk size
NDC = D // DC      # d chunks (16)
NDM = DC // 128    # d tiles per chunk (4)

# "f32r": store logits operands in f32, run matmul as float32r (full rate).
# "bf16": store logits operands as bf16.
LOGITS_DT = "f32r"

AF = mybir.ActivationFunctionType
ALU = mybir.AluOpType


LS = F32R if LOGITS_DT == "f32r" else BF16


def build_body(ctx: ExitStack, tc: tile.TileContext, nc, Qd, Cd, Wd, bd, Od):
    singles = ctx.enter_context(tc.tile_pool(name="singles", bufs=1))
    qstat = ctx.enter_context(tc.tile_pool(name="qstat", bufs=1))
    cpool = ctx.enter_context(tc.tile_pool(name="cpool", bufs=4))
    ctpool = ctx.enter_context(tc.tile_pool(name="ctp", bufs=3))
    exppool = ctx.enter_context(tc.tile_pool(name="expp", bufs=2))
    outpool = ctx.enter_context(tc.tile_pool(name="outp", bufs=3))
    small = ctx.enter_context(tc.tile_pool(name="small", bufs=4))
    psum_l = ctx.enter_context(tc.tile_pool(name="psl", bufs=2, space="PSUM"))
    psum_t = ctx.enter_context(tc.tile_pool(name="pst", bufs=2, space="PSUM"))
    psum_a = ctx.enter_context(tc.tile_pool(name="psa", bufs=1, space="PSUM"))

    ident = singles.tile([128, 128], F32)
    make_identity(nc, ident)

    # --- main loop over d chunks, software-pipelined C prep ---
    def c_load(dc):
        c_nat = cpool.tile([128, NDM, H], F32, tag="cnat", name=f"cnat{dc}")
        nc.sync.dma_start(
            out=c_nat[:],
            in_=Cd[dc * DC:(dc + 1) * DC, :].rearrange("(a p) h -> p a h", p=128))
        return c_nat

    def c_transpose(dc, c_nat, hc):
        pt = psum_t.tile([128, 512], F32, tag="pt", name=f"ptc{dc}_{hc}")
        for dm in range(NDM):
            nc.tensor.transpose(
                pt[:, dm * 128:(dm + 1) * 128],
                c_nat[:, dm, hc * 128:(hc + 1) * 128], ident[:])
        return pt

    c_nats = {}
    cts = {}

    def c_prep(dc):
        cts[dc] = ctpool.tile([128, HC, DC], LS, tag="ct", name=f"ct{dc}")
        for hc in range(HC):
            pt = c_transpose(dc, c_nats[dc], hc)
            nc.vector.tensor_copy(cts[dc][:, hc, :], pt[:])

    # --- W^T [h, o] and bias ---
    w_nat = singles.tile([128, HC, H], F32)  # [o in-chunk, om, h]
    for om in range(HC):
        nc.sync.dma_start(out=w_nat[:, om, :], in_=Wd[om * 128:(om + 1) * 128, :])
    wt = qstat.tile([128, HC, H], LS)  # [h in-chunk, hc, o]
    for om in range(HC):
        for hc in range(HC):
            pt = psum_t.tile([128, 128], F32)
            nc.tensor.transpose(pt[:], w_nat[:, om, hc * 128:(hc + 1) * 128], ident[:])
            nc.vector.tensor_copy(wt[:, hc, om * 128:(om + 1) * 128], pt[:])
    bias = singles.tile([128, HC, 1], F32)
    nc.sync.dma_start(out=bias[:, :, 0], in_=bd.rearrange("(c p) -> p c", p=128))
    for _dc in range(3):
        c_nats[_dc] = c_load(_dc)
    c_prep(0)

    # --- Q^T [h, q] ---
    qt = qstat.tile([128, HC, QL], LS)
    q_nat = cpool.tile([128, NQT, H], F32, tag="qnat")
    q_view = Qd.rearrange("(a p) h -> p a h", p=128)
    for qg in range(NQT // 4):
        nc.sync.dma_start(out=q_nat[:, qg * 4:(qg + 1) * 4, :],
                          in_=q_view[:, qg * 4:(qg + 1) * 4, :])
    for qg in range(NQT // 4):
        for hc in range(HC):
            pt = psum_t.tile([128, 512], F32)
            for k in range(4):
                qi = qg * 4 + k
                nc.tensor.transpose(
                    pt[:, k * 128:(k + 1) * 128],
                    q_nat[:, qi, hc * 128:(hc + 1) * 128], ident[:])
            nc.vector.tensor_copy(qt[:, hc, qg * 512:(qg + 1) * 512], pt[:])

    c_prep(1)

    # --- QpT = gelu(W Q^T + b) [o, q] ---
    qpt = qstat.tile([128, HC, QL], LS)
    for om in range(HC):
        for nn in range(QL // 512):
            pl = psum_l.tile([128, 512], F32)
            for hc in range(HC):
                nc.tensor.matmul(
                    pl[:],
                    wt[:, hc, om * 128:(om + 1) * 128],
                    qt[:, hc, nn * 512:(nn + 1) * 512],
                    start=(hc == 0),
                    stop=(hc == HC - 1),
                )
            nc.scalar.activation(
                qpt[:, om, nn * 512:(nn + 1) * 512], pl[:], AF.Gelu,
                bias=bias[:, om, :], scale=1.0,
            )

    # --- Qp natural bf16 [q, o] with ones column ---
    qp = qstat.tile([128, NQT, H + 1], BF16)
    for qg in range(NQT // 4):
        for om in range(HC):
            pt = psum_t.tile([128, 512], F32)
            for k in range(4):
                qi = qg * 4 + k
                src = qpt[:, om, qi * 128:(qi + 1) * 128]
                if LOGITS_DT == "f32r":
                    src = src.bitcast(F32)
                nc.tensor.transpose(pt[:, k * 128:(k + 1) * 128], src, ident[:])
            nc.vector.tensor_copy(
                qp[:, qg * 4:(qg + 1) * 4, om * 128:(om + 1) * 128],
                pt.rearrange("p (a b) -> p a b", a=4))
    nc.vector.memset(qp[:, :, H:H + 1], 1.0)

    # Lag the attended matmuls two q-tiles behind logits+exp so the PE
    # never waits on the ACT exp latency.
    LAG = 2
    for dc in range(NDC):
        c_nat, ct = c_nats[dc], cts[dc]
        expt = exppool.tile([128, NQT, DC], BF16)
        pa = [psum_a.tile([128, H + 1], F32, tag=f"a{dm}", name=f"pa{dm}")
              for dm in range(NDM)]
        nxt = dc + 2
        for step in range(NQT + LAG):
            if step == 2 and dc + 3 < NDC:
                c_nats[dc + 3] = c_load(dc + 3)
            if step == 6 and nxt < NDC:
                cts[nxt] = ctpool.tile([128, HC, DC], LS, tag="ct", name=f"ct{nxt}")
            if step in (8, 11) and nxt < NDC:
                hc = 0 if step == 8 else 1
                pt = c_transpose(nxt, c_nats[nxt], hc)
                nc.vector.tensor_copy(cts[nxt][:, hc, :], pt[:])
            if step < NQT:
                qi = step
                pl = psum_l.tile([128, DC], F32)
                for hc in range(HC):
                    nc.tensor.matmul(
                        pl[:],
                        qpt[:, hc, qi * 128:(qi + 1) * 128],
                        ct[:, hc, :],
                        start=(hc == 0),
                        stop=(hc == HC - 1),
                    )
                nc.scalar.activation(expt[:, qi, :], pl[:], AF.Exp)
            if step >= LAG:
                qj = step - LAG
                for dm in range(NDM):
                    nc.tensor.matmul(
                        pa[dm][:],
                        expt[:, qj, dm * 128:(dm + 1) * 128],
                        qp[:, qj, :],
                        start=(qj == 0),
                        stop=(qj == NQT - 1),
                    )

        o_sb = outpool.tile([128, NDM, H], F32)
        for dm in range(NDM):
            rec = small.tile([128, 1], F32)
            nc.vector.reciprocal(rec[:], pa[dm][:, H:H + 1])
            nc.vector.scalar_tensor_tensor(
                o_sb[:, dm, :], pa[dm][:, 0:H], rec[:], c_nat[:, dm, :],
                ALU.mult, ALU.add,
            )
        nc.sync.dma_start(
            out=Od[dc * DC:(dc + 1) * DC, :].rearrange("(a p) h -> p a h", p=128),
            in_=o_sb[:])
        del c_nats[dc], cts[dc]


def build_nc():
    nc = bacc.Bacc("TRN2", target_bir_lowering=False, debug=False,
                   num_devices=N_CORES)
    Qd = nc.dram_tensor("Q", [QL, H], F32, kind="ExternalInput")
    Cd = nc.dram_tensor("C", [D, H], F32, kind="ExternalInput")
    Wd = nc.dram_tensor("W", [H, H], F32, kind="ExternalInput")
    bd = nc.dram_tensor("b", [H], F32, kind="ExternalInput")
    Od = nc.dram_tensor("out", [D, H], F32, kind="ExternalOutput")
    with tile.TileContext(nc) as tc:
        with ExitStack() as ctx:
            build_body(ctx, tc, nc, Qd[:], Cd[:], Wd[:], bd[:], Od[:])
    nc.finalize()
    return nc


_NC = None


def get_nc():
    global _NC
    if _NC is None:
        _NC = build_nc()
    return _NC


def kernel(Q, C, W, b):
    assert Q.shape == (B, QL, H) and C.shape == (B, D, H)
    nc = get_nc()
    in_maps = [
        {
            "Q": np.ascontiguousarray(Q[i], dtype=np.float32),
            "C": np.ascontiguousarray(C[i], dtype=np.float32),
            "W": np.ascontiguousarray(W, dtype=np.float32),
            "b": np.ascontiguousarray(b, dtype=np.float32),
        }
        for i in range(N_CORES)
    ]
    res = run_bass_kernel_spmd(nc, in_maps, core_ids=list(range(N_CORES)))
    return np.stack([res.results[i]["out"] for i in range(N_CORES)], axis=0)
